# revision 10
# baseline (speedup 1.0000x reference)
"""AngularLoss Trainium2 kernel (8 NeuronCores, SPMD data-parallel).

Computation (reference):
    t2  = tan(alpha_deg * pi/180)^2
    apn = rowsum((a + p) * n)          # [N,1]
    ap  = rowsum(a * p)                # [N,1]
    f   = 4*t2*apn - 2*(1+t2)*ap       # [N,1]
    out = logsumexp(f, axis=0)         # [1]

Strategy: shard N=262144 rows across 8 cores (32768 rows each).  Each core
streams its 3x16MB shard through SBUF in 16 steps, computing per-row dots
with a DVE/ACT/GPSIMD engine split, and reduces its 32768 f-values to a
per-partition (max, sum-exp) pair [128,2].  Host combines the 8x128 partial
(m, s) pairs into the final logsumexp — no on-chip collective needed.

Engine split per step (tile [128 part, 16 rows, 128 d] per tensor):
    SP(sync): HWDGE DMA loads (1MB each)
    GPSIMD:   q = a + p
    DVE:      t = q*n, u = a*p, grouped reduce(t) -> apn columns
    ACT:      reduce(u) -> ap columns (activation Copy + accum_out)
"""

import numpy as np

import concourse.bacc as bacc
import concourse.bass as bass
import concourse.tile as tile
from concourse import mybir
from concourse.bass_utils import run_bass_kernel_spmd

N, D = 262144, 128
NCORES = 8
N_LOCAL = N // NCORES            # 32768 rows per core
P = 128                          # partitions
ROWS_PER_PART = N_LOCAL // P     # 256 rows owned by each partition
B = 16                           # rows (per partition) processed per step
STEPS = ROWS_PER_PART // B       # 16
ACT_ROWS = 6                     # rows per product per step reduced on ACT
F32 = mybir.dt.float32
BF16 = mybir.dt.bfloat16
CDT = BF16                       # on-chip compute dtype (DMA casts f32->bf16)


def _build(c1: float, c2: float) -> bass.Bass:
    nc = bacc.Bacc()
    a_ext = nc.declare_dram_parameter("anchor", [N_LOCAL, D], F32, isOutput=False)
    p_ext = nc.declare_dram_parameter("positive", [N_LOCAL, D], F32, isOutput=False)
    n_ext = nc.declare_dram_parameter("negative", [N_LOCAL, D], F32, isOutput=False)
    out_ext = nc.declare_dram_parameter("out", [P, 2], F32, isOutput=True)

    # Partition p owns rows [p*256, (p+1)*256): contiguous 128KB per partition
    # in DRAM -> fully coalesced DMA descriptors.
    a_v = a_ext.rearrange("(p r) d -> p r d", p=P)
    p_v = p_ext.rearrange("(p r) d -> p r d", p=P)
    n_v = n_ext.rearrange("(p r) d -> p r d", p=P)

    with tile.TileContext(nc) as tc:
        with (
            tc.tile_pool(name="ina", bufs=3) as pool_a,
            tc.tile_pool(name="inp", bufs=3) as pool_p,
            tc.tile_pool(name="inn", bufs=3) as pool_n,
            tc.tile_pool(name="q", bufs=2) as pool_q,
            tc.tile_pool(name="t", bufs=2) as pool_t,
            tc.tile_pool(name="u", bufs=2) as pool_u,
            tc.tile_pool(name="acc", bufs=1) as pool_acc,
        ):
            apn = pool_acc.tile([P, ROWS_PER_PART], F32)
            ap = pool_acc.tile([P, ROWS_PER_PART], F32)
            dummy = pool_acc.tile([P, D], CDT)  # ACT reduce writes land here

            for j in range(STEPS):
                ta = pool_a.tile([P, B * D], CDT)
                tp = pool_p.tile([P, B * D], CDT)
                tn = pool_n.tile([P, B * D], CDT)
                rows = slice(j * B, (j + 1) * B)
                # SWDGE (gpsimd) DMA casts f32 -> bf16 in the datapath.
                nc.gpsimd.dma_start(out=ta[:], in_=a_v[:, rows, :])
                nc.gpsimd.dma_start(out=tp[:], in_=p_v[:, rows, :])
                nc.gpsimd.dma_start(out=tn[:], in_=n_v[:, rows, :])

                tq = pool_q.tile([P, B * D], CDT)
                nc.vector.tensor_tensor(tq[:], ta[:], tp[:], mybir.AluOpType.add)

                tt = pool_t.tile([P, B * D], CDT)
                tu = pool_u.tile([P, B * D], CDT)
                nc.vector.tensor_tensor(tt[:], tq[:], tn[:], mybir.AluOpType.mult)
                nc.vector.tensor_tensor(tu[:], ta[:], tp[:], mybir.AluOpType.mult)

                # per-row sums: bulk on DVE (grouped reduce); last ACT_ROWS
                # rows of each product on ACT (copy+accum) to balance load.
                BT = B - ACT_ROWS
                nc.vector.tensor_reduce(
                    out=apn[:, j * B : j * B + BT],
                    in_=tt[:, : BT * D].rearrange("p (r d) -> p r d", d=D),
                    axis=mybir.AxisListType.X,
                    op=mybir.AluOpType.add,
                )
                nc.vector.tensor_reduce(
                    out=ap[:, j * B : j * B + BT],
                    in_=tu[:, : BT * D].rearrange("p (r d) -> p r d", d=D),
                    axis=mybir.AxisListType.X,
                    op=mybir.AluOpType.add,
                )
                for dst, src in ((apn, tt), (ap, tu)):
                    for r in range(BT, B):
                        col = j * B + r
                        nc.scalar.activation(
                            out=dummy[:],
                            in_=src[:, r * D : (r + 1) * D],
                            func=mybir.ActivationFunctionType.Copy,
                            accum_out=dst[:, col : col + 1],
                        )

            # f = c1*apn + c2*ap
            tmp = pool_acc.tile([P, ROWS_PER_PART], F32)
            f = pool_acc.tile([P, ROWS_PER_PART], F32)
            nc.vector.tensor_scalar_mul(tmp[:], ap[:], c2)
            nc.vector.scalar_tensor_tensor(
                f[:], apn[:], c1, tmp[:],
                op0=mybir.AluOpType.mult, op1=mybir.AluOpType.add,
            )

            # per-partition logsumexp partials: m = rowmax(f), s = sum(exp(f-m))
            m = pool_acc.tile([P, 1], F32)
            negm = pool_acc.tile([P, 1], F32)
            s = pool_acc.tile([P, 1], F32)
            expf = pool_acc.tile([P, ROWS_PER_PART], F32)
            nc.vector.tensor_reduce(
                out=m[:], in_=f[:], axis=mybir.AxisListType.X, op=mybir.AluOpType.max
            )
            nc.vector.tensor_scalar_mul(negm[:], m[:], -1.0)
            nc.scalar.activation(
                out=expf[:],
                in_=f[:],
                func=mybir.ActivationFunctionType.Exp,
                bias=negm[:],
                scale=1.0,
                accum_out=s[:],
            )
            nc.sync.dma_start(out=out_ext[:, 0:1], in_=m[:])
            nc.sync.dma_start(out=out_ext[:, 1:2], in_=s[:])
    nc.compile()
    return nc


def kernel(anchor, positive, negative, alpha):
    anchor = np.ascontiguousarray(np.asarray(anchor, dtype=np.float32))
    positive = np.ascontiguousarray(np.asarray(positive, dtype=np.float32))
    negative = np.ascontiguousarray(np.asarray(negative, dtype=np.float32))
    a_rad = 2.0 * np.pi * float(np.asarray(alpha)) / 360.0
    t2 = float(np.tan(a_rad) ** 2)
    c1 = 4.0 * t2
    c2 = -2.0 * (1.0 + t2)

    nc = _build(c1, c2)
    in_maps = []
    for i in range(NCORES):
        sl = slice(i * N_LOCAL, (i + 1) * N_LOCAL)
        in_maps.append(
            {"anchor": anchor[sl], "positive": positive[sl], "negative": negative[sl]}
        )
    res = run_bass_kernel_spmd(nc, in_maps, core_ids=list(range(NCORES)))

    ms = np.concatenate([np.asarray(r["out"]) for r in res.results], axis=0)
    m = ms[:, 0].astype(np.float64)
    s = ms[:, 1].astype(np.float64)
    M = m.max()
    S = np.sum(s * np.exp(m - M))
    return np.array([np.log(S) + M], dtype=np.float32)


if __name__ == "__main__":
    rng = np.random.default_rng(0)
    out = kernel(
        anchor=rng.standard_normal((N, D), dtype=np.float32),
        positive=rng.standard_normal((N, D), dtype=np.float32),
        negative=rng.standard_normal((N, D), dtype=np.float32),
        alpha=np.int64(45),
    )
    print("kernel out:", out)


# revision 12
# speedup vs baseline: 1.1235x; 1.1235x over previous
"""AngularLoss Trainium2 kernel (8 NeuronCores, SPMD data-parallel).

Computation (reference):
    t2  = tan(alpha_deg * pi/180)^2
    apn = rowsum((a + p) * n)          # [N,1]
    ap  = rowsum(a * p)                # [N,1]
    f   = 4*t2*apn - 2*(1+t2)*ap       # [N,1]
    out = logsumexp(f, axis=0)         # [1]

Strategy: shard N=262144 rows across 8 cores (32768 rows each).  Each core
streams its 3x16MB shard through SBUF in 16 steps, computing per-row dots
with a DVE/ACT/GPSIMD engine split, and reduces its 32768 f-values to a
per-partition (max, sum-exp) pair [128,2].  Host combines the 8x128 partial
(m, s) pairs into the final logsumexp — no on-chip collective needed.

Engine split per step (tile [128 part, 16 rows, 128 d] per tensor):
    SP(sync): HWDGE DMA loads (1MB each)
    GPSIMD:   q = a + p
    DVE:      t = q*n, u = a*p, grouped reduce(t) -> apn columns
    ACT:      reduce(u) -> ap columns (activation Copy + accum_out)
"""

import numpy as np

import concourse.bacc as bacc
import concourse.bass as bass
import concourse.tile as tile
from concourse import mybir
from concourse.bass_utils import run_bass_kernel_spmd

N, D = 262144, 128
NCORES = 8
N_LOCAL = N // NCORES            # 32768 rows per core
P = 128                          # partitions
ROWS_PER_PART = N_LOCAL // P     # 256 rows owned by each partition
B = 16                           # rows (per partition) processed per step
STEPS = ROWS_PER_PART // B       # 16
ACT_ROWS = 5                     # rows per product per step reduced on ACT
F32 = mybir.dt.float32
BF16 = mybir.dt.bfloat16
CDT = BF16                       # on-chip compute dtype (DMA casts f32->bf16)


def _build(c1: float, c2: float) -> bass.Bass:
    nc = bacc.Bacc()
    a_ext = nc.declare_dram_parameter("anchor", [N_LOCAL, D], F32, isOutput=False)
    p_ext = nc.declare_dram_parameter("positive", [N_LOCAL, D], F32, isOutput=False)
    n_ext = nc.declare_dram_parameter("negative", [N_LOCAL, D], F32, isOutput=False)
    out_ext = nc.declare_dram_parameter("out", [P, 2], F32, isOutput=True)

    # Partition p owns rows [p*256, (p+1)*256): contiguous 128KB per partition
    # in DRAM -> fully coalesced DMA descriptors.
    a_v = a_ext.rearrange("(p r) d -> p r d", p=P)
    p_v = p_ext.rearrange("(p r) d -> p r d", p=P)
    n_v = n_ext.rearrange("(p r) d -> p r d", p=P)

    with tile.TileContext(nc) as tc:
        with (
            tc.tile_pool(name="ina", bufs=4) as pool_a,
            tc.tile_pool(name="inp", bufs=4) as pool_p,
            tc.tile_pool(name="inn", bufs=4) as pool_n,
            tc.tile_pool(name="q", bufs=3) as pool_q,
            tc.tile_pool(name="t", bufs=3) as pool_t,
            tc.tile_pool(name="u", bufs=3) as pool_u,
            tc.tile_pool(name="acc", bufs=1) as pool_acc,
        ):
            apn = pool_acc.tile([P, ROWS_PER_PART], F32)
            ap = pool_acc.tile([P, ROWS_PER_PART], F32)
            dummy = pool_acc.tile([P, D], CDT)  # ACT reduce writes land here

            for j in range(STEPS):
                ta = pool_a.tile([P, B * D], CDT)
                tp = pool_p.tile([P, B * D], CDT)
                tn = pool_n.tile([P, B * D], CDT)
                rows = slice(j * B, (j + 1) * B)
                # SWDGE (gpsimd) DMA casts f32 -> bf16 in the datapath.
                nc.gpsimd.dma_start(out=ta[:], in_=a_v[:, rows, :])
                nc.gpsimd.dma_start(out=tp[:], in_=p_v[:, rows, :])
                nc.gpsimd.dma_start(out=tn[:], in_=n_v[:, rows, :])

                tq = pool_q.tile([P, B * D], CDT)
                nc.vector.tensor_tensor(tq[:], ta[:], tp[:], mybir.AluOpType.add)

                tt = pool_t.tile([P, B * D], CDT)
                tu = pool_u.tile([P, B * D], CDT)
                nc.vector.tensor_tensor(tt[:], tq[:], tn[:], mybir.AluOpType.mult)
                nc.vector.tensor_tensor(tu[:], ta[:], tp[:], mybir.AluOpType.mult)

                # per-row sums: bulk on DVE (grouped reduce); last ACT_ROWS
                # rows of each product on ACT (copy+accum) to balance load.
                BT = B - ACT_ROWS
                nc.vector.tensor_reduce(
                    out=apn[:, j * B : j * B + BT],
                    in_=tt[:, : BT * D].rearrange("p (r d) -> p r d", d=D),
                    axis=mybir.AxisListType.X,
                    op=mybir.AluOpType.add,
                )
                nc.vector.tensor_reduce(
                    out=ap[:, j * B : j * B + BT],
                    in_=tu[:, : BT * D].rearrange("p (r d) -> p r d", d=D),
                    axis=mybir.AxisListType.X,
                    op=mybir.AluOpType.add,
                )
                for dst, src in ((apn, tt), (ap, tu)):
                    for r in range(BT, B):
                        col = j * B + r
                        nc.scalar.activation(
                            out=dummy[:],
                            in_=src[:, r * D : (r + 1) * D],
                            func=mybir.ActivationFunctionType.Copy,
                            accum_out=dst[:, col : col + 1],
                        )

            # f = c1*apn + c2*ap
            tmp = pool_acc.tile([P, ROWS_PER_PART], F32)
            f = pool_acc.tile([P, ROWS_PER_PART], F32)
            nc.vector.tensor_scalar_mul(tmp[:], ap[:], c2)
            nc.vector.scalar_tensor_tensor(
                f[:], apn[:], c1, tmp[:],
                op0=mybir.AluOpType.mult, op1=mybir.AluOpType.add,
            )

            # per-partition logsumexp partials: m = rowmax(f), s = sum(exp(f-m))
            m = pool_acc.tile([P, 1], F32)
            negm = pool_acc.tile([P, 1], F32)
            s = pool_acc.tile([P, 1], F32)
            expf = pool_acc.tile([P, ROWS_PER_PART], F32)
            nc.vector.tensor_reduce(
                out=m[:], in_=f[:], axis=mybir.AxisListType.X, op=mybir.AluOpType.max
            )
            nc.vector.tensor_scalar_mul(negm[:], m[:], -1.0)
            nc.scalar.activation(
                out=expf[:],
                in_=f[:],
                func=mybir.ActivationFunctionType.Exp,
                bias=negm[:],
                scale=1.0,
                accum_out=s[:],
            )
            nc.sync.dma_start(out=out_ext[:, 0:1], in_=m[:])
            nc.sync.dma_start(out=out_ext[:, 1:2], in_=s[:])
    nc.compile()
    return nc


def kernel(anchor, positive, negative, alpha):
    anchor = np.ascontiguousarray(np.asarray(anchor, dtype=np.float32))
    positive = np.ascontiguousarray(np.asarray(positive, dtype=np.float32))
    negative = np.ascontiguousarray(np.asarray(negative, dtype=np.float32))
    a_rad = 2.0 * np.pi * float(np.asarray(alpha)) / 360.0
    t2 = float(np.tan(a_rad) ** 2)
    c1 = 4.0 * t2
    c2 = -2.0 * (1.0 + t2)

    nc = _build(c1, c2)
    in_maps = []
    for i in range(NCORES):
        sl = slice(i * N_LOCAL, (i + 1) * N_LOCAL)
        in_maps.append(
            {"anchor": anchor[sl], "positive": positive[sl], "negative": negative[sl]}
        )
    res = run_bass_kernel_spmd(nc, in_maps, core_ids=list(range(NCORES)))

    ms = np.concatenate([np.asarray(r["out"]) for r in res.results], axis=0)
    m = ms[:, 0].astype(np.float64)
    s = ms[:, 1].astype(np.float64)
    M = m.max()
    S = np.sum(s * np.exp(m - M))
    return np.array([np.log(S) + M], dtype=np.float32)


if __name__ == "__main__":
    rng = np.random.default_rng(0)
    out = kernel(
        anchor=rng.standard_normal((N, D), dtype=np.float32),
        positive=rng.standard_normal((N, D), dtype=np.float32),
        negative=rng.standard_normal((N, D), dtype=np.float32),
        alpha=np.int64(45),
    )
    print("kernel out:", out)


# revision 15
# speedup vs baseline: 1.1813x; 1.0515x over previous
"""AngularLoss Trainium2 kernel (8 NeuronCores, SPMD data-parallel).

Computation (reference):
    t2  = tan(alpha_deg * pi/180)^2
    apn = rowsum((a + p) * n)          # [N,1]
    ap  = rowsum(a * p)                # [N,1]
    f   = 4*t2*apn - 2*(1+t2)*ap       # [N,1]
    out = logsumexp(f, axis=0)         # [1]

Strategy: shard N=262144 rows across 8 cores (32768 rows each).  Each core
streams its 3x16MB shard through SBUF in 16 steps, computing per-row dots
with a DVE/ACT/GPSIMD engine split, and reduces its 32768 f-values to a
per-partition (max, sum-exp) pair [128,2].  Host combines the 8x128 partial
(m, s) pairs into the final logsumexp — no on-chip collective needed.

Engine split per step (tile [128 part, 16 rows, 128 d] per tensor):
    SP(sync): HWDGE DMA loads (1MB each)
    GPSIMD:   q = a + p
    DVE:      t = q*n, u = a*p, grouped reduce(t) -> apn columns
    ACT:      reduce(u) -> ap columns (activation Copy + accum_out)
"""

import numpy as np

import concourse.bacc as bacc
import concourse.bass as bass
import concourse.tile as tile
from concourse import mybir
from concourse.bass_utils import run_bass_kernel_spmd

N, D = 262144, 128
NCORES = 8
N_LOCAL = N // NCORES            # 32768 rows per core
P = 128                          # partitions
ROWS_PER_PART = N_LOCAL // P     # 256 rows owned by each partition
B = 16                           # rows (per partition) per compute sub-step
B_DMA = 16                       # rows (per partition) per DMA load
STEPS = ROWS_PER_PART // B       # 16
ACT_ROWS = 2                     # rows per product per sub-step reduced on ACT
F32 = mybir.dt.float32
BF16 = mybir.dt.bfloat16
CDT = BF16                       # on-chip compute dtype (DMA casts f32->bf16)


def _build(c1: float, c2: float) -> bass.Bass:
    nc = bacc.Bacc()
    a_ext = nc.declare_dram_parameter("anchor", [N_LOCAL, D], F32, isOutput=False)
    p_ext = nc.declare_dram_parameter("positive", [N_LOCAL, D], F32, isOutput=False)
    n_ext = nc.declare_dram_parameter("negative", [N_LOCAL, D], F32, isOutput=False)
    out_ext = nc.declare_dram_parameter("out", [P, 2], F32, isOutput=True)

    # Partition p owns rows [p*256, (p+1)*256): contiguous 128KB per partition
    # in DRAM -> fully coalesced DMA descriptors.
    a_v = a_ext.rearrange("(p r) d -> p r d", p=P)
    p_v = p_ext.rearrange("(p r) d -> p r d", p=P)
    n_v = n_ext.rearrange("(p r) d -> p r d", p=P)

    with tile.TileContext(nc) as tc:
        with (
            tc.tile_pool(name="ina", bufs=3) as pool_a,
            tc.tile_pool(name="inp", bufs=3) as pool_p,
            tc.tile_pool(name="inn", bufs=3) as pool_n,
            tc.tile_pool(name="q", bufs=3) as pool_q,
            tc.tile_pool(name="t", bufs=3) as pool_t,
            tc.tile_pool(name="u", bufs=3) as pool_u,
            tc.tile_pool(name="fold", bufs=3) as pool_f,
            tc.tile_pool(name="acc", bufs=1) as pool_acc,
        ):
            apn = pool_acc.tile([P, ROWS_PER_PART], F32)
            ap = pool_acc.tile([P, ROWS_PER_PART], F32)
            dummy = pool_acc.tile([P, D], CDT)  # ACT reduce writes land here

            for jd in range(ROWS_PER_PART // B_DMA):
                ta = pool_a.tile([P, B_DMA * D], CDT)
                tp = pool_p.tile([P, B_DMA * D], CDT)
                tn = pool_n.tile([P, B_DMA * D], CDT)
                rows = slice(jd * B_DMA, (jd + 1) * B_DMA)
                # SWDGE (gpsimd) DMA casts f32 -> bf16 in the datapath.
                nc.gpsimd.dma_start(out=ta[:], in_=a_v[:, rows, :])
                nc.gpsimd.dma_start(out=tp[:], in_=p_v[:, rows, :])
                nc.gpsimd.dma_start(out=tn[:], in_=n_v[:, rows, :])

                for js in range(B_DMA // B):
                    j = jd * (B_DMA // B) + js
                    sub = slice(js * B * D, (js + 1) * B * D)
                    sta, stp, stn = ta[:, sub], tp[:, sub], tn[:, sub]

                    tq = pool_q.tile([P, B * D], CDT)
                    nc.vector.tensor_tensor(tq[:], sta, stp, mybir.AluOpType.add)

                    tt = pool_t.tile([P, B * D], CDT)
                    tu = pool_u.tile([P, B * D], CDT)
                    nc.vector.tensor_tensor(tt[:], tq[:], stn, mybir.AluOpType.mult)
                    nc.vector.tensor_tensor(tu[:], sta, stp, mybir.AluOpType.mult)

                    # Per-row sums. Bulk rows on DVE: fold halves (bf16 TT,
                    # 2x mode) then grouped reduce; last ACT_ROWS rows of
                    # each product go to ACT (copy+accum).
                    BT = B - ACT_ROWS
                    for dst, src in ((apn, tt), (ap, tu)):
                        v3 = src[:, : BT * D].rearrange("p (r d) -> p r d", d=D)
                        tf = pool_f.tile([P, BT * (D // 2)], CDT, tag="fold")
                        nc.vector.tensor_tensor(
                            tf[:],
                            v3[:, :, : D // 2],
                            v3[:, :, D // 2 :],
                            mybir.AluOpType.add,
                        )
                        nc.vector.tensor_reduce(
                            out=dst[:, j * B : j * B + BT],
                            in_=tf[:].rearrange("p (r d) -> p r d", d=D // 2),
                            axis=mybir.AxisListType.X,
                            op=mybir.AluOpType.add,
                        )
                        for r in range(BT, B):
                            col = j * B + r
                            nc.scalar.activation(
                                out=dummy[:],
                                in_=src[:, r * D : (r + 1) * D],
                                func=mybir.ActivationFunctionType.Copy,
                                accum_out=dst[:, col : col + 1],
                            )

            # f = c1*apn + c2*ap
            tmp = pool_acc.tile([P, ROWS_PER_PART], F32)
            f = pool_acc.tile([P, ROWS_PER_PART], F32)
            nc.vector.tensor_scalar_mul(tmp[:], ap[:], c2)
            nc.vector.scalar_tensor_tensor(
                f[:], apn[:], c1, tmp[:],
                op0=mybir.AluOpType.mult, op1=mybir.AluOpType.add,
            )

            # per-partition logsumexp partials: m = rowmax(f), s = sum(exp(f-m))
            m = pool_acc.tile([P, 1], F32)
            negm = pool_acc.tile([P, 1], F32)
            s = pool_acc.tile([P, 1], F32)
            expf = pool_acc.tile([P, ROWS_PER_PART], F32)
            nc.vector.tensor_reduce(
                out=m[:], in_=f[:], axis=mybir.AxisListType.X, op=mybir.AluOpType.max
            )
            nc.vector.tensor_scalar_mul(negm[:], m[:], -1.0)
            nc.scalar.activation(
                out=expf[:],
                in_=f[:],
                func=mybir.ActivationFunctionType.Exp,
                bias=negm[:],
                scale=1.0,
                accum_out=s[:],
            )
            nc.sync.dma_start(out=out_ext[:, 0:1], in_=m[:])
            nc.sync.dma_start(out=out_ext[:, 1:2], in_=s[:])
    nc.compile()
    return nc


def kernel(anchor, positive, negative, alpha):
    anchor = np.ascontiguousarray(np.asarray(anchor, dtype=np.float32))
    positive = np.ascontiguousarray(np.asarray(positive, dtype=np.float32))
    negative = np.ascontiguousarray(np.asarray(negative, dtype=np.float32))
    a_rad = 2.0 * np.pi * float(np.asarray(alpha)) / 360.0
    t2 = float(np.tan(a_rad) ** 2)
    c1 = 4.0 * t2
    c2 = -2.0 * (1.0 + t2)

    nc = _build(c1, c2)
    in_maps = []
    for i in range(NCORES):
        sl = slice(i * N_LOCAL, (i + 1) * N_LOCAL)
        in_maps.append(
            {"anchor": anchor[sl], "positive": positive[sl], "negative": negative[sl]}
        )
    res = run_bass_kernel_spmd(nc, in_maps, core_ids=list(range(NCORES)))

    ms = np.concatenate([np.asarray(r["out"]) for r in res.results], axis=0)
    m = ms[:, 0].astype(np.float64)
    s = ms[:, 1].astype(np.float64)
    M = m.max()
    S = np.sum(s * np.exp(m - M))
    return np.array([np.log(S) + M], dtype=np.float32)


if __name__ == "__main__":
    rng = np.random.default_rng(0)
    out = kernel(
        anchor=rng.standard_normal((N, D), dtype=np.float32),
        positive=rng.standard_normal((N, D), dtype=np.float32),
        negative=rng.standard_normal((N, D), dtype=np.float32),
        alpha=np.int64(45),
    )
    print("kernel out:", out)
